# revision 1
# baseline (speedup 1.0000x reference)
"""Trainium2 Bass kernel for the O2O classification head (GNN message passing).

Strategy
--------
The reference edge tensor is rank-structured:
    edge[b,i,j,:] = (f_in_i + pos_i@W_pos + b_in + b_pos) - (f_out_j + b_out + pos_j@W_pos)
                  = A_i - C_j
so after the first edge MLP layer the pre-gelu values are p_i - q_j + b_e1 with
p = A@W_e1, q = C@W_e1 computed once per node.  The [B,N,N,128] edge tensor is
never materialized; each (i,j) pair costs one 128-wide gelu + dot with W_e2.

Host-side, nodes are sorted by (cls desc, id desc).  Then
    suppress[i,j] != 0  requires  rank_i < rank_j
so for a j-tile only the i-prefix [0, rank_max) contributes; everything else is
masked to zero exactly as in the reference (the max always sees explicit zeros,
e.g. at i == j).

Sharding: 2 cores per batch.  Each core takes the 32-wide j-blocks of one
parity (global block 2t+P for t = 0..7) with i-prefix length 64*(t+1) — every
core runs an identical program; all per-core variation is input data.
"""

import sys
import numpy as np

if "/opt/trn_rl_repo" not in sys.path:
    sys.path.insert(0, "/opt/trn_rl_repo")

B, N = 4, 512
H_DIM, I_DIM = 64, 128
N_CORES = 8
N_TILES = 8          # j-tiles per core, 32 j's each
TJ = 32              # j's per tile
ILEN = [64 * (t + 1) for t in range(N_TILES)]   # i-prefix per tile
F32 = np.float32

IMG_W, IMG_H, CENTER_H = 800.0, 320.0, 160.0
NUM_OFFSETS = 72
CONF_THRES = 0.4

_PROGRAM = None  # cached compiled program

INPUT_SPECS = [
    ("bfT_i", (H_DIM, N)),
    ("posT_i", (2, N)),
    ("bfT_j", (H_DIM, 256)),
    ("posT_j", (2, 256)),
    ("angrow", (1, N)),
    ("angcol", (128, 2)),
    ("rankcol", (128, 2)),
    ("iota", (1, N)),
    ("we2d", (128, 32 * 32)),
    ("cls_loc", (1, 256)),
    ("W_cls", (64, 64)),
    ("bcls", (64, 1)),
    ("W_in", (64, 128)),
    ("W_out", (64, 128)),
    ("W_pos", (2, 128)),
    ("bpos", (128, 1)),
    ("W_e1", (128, 128)),
    ("be1", (128, 1)),
    ("we2", (128, 1)),
    ("be2c", (128, 1)),
    ("W_n1", (1, 64)),
    ("bn1", (64, 1)),
    ("W_n2", (64, 64)),
    ("bn2", (64, 1)),
    ("W_head", (64, 1)),
    ("bh", (1, 1)),
]


def _build_program(stage=99, num_devices=N_CORES):
    import contextlib
    import concourse.bass as bass  # noqa: F401
    import concourse.tile as tile
    from concourse import bacc, mybir

    f32 = mybir.dt.float32
    AF = mybir.ActivationFunctionType
    OP = mybir.AluOpType
    AX = mybir.AxisListType

    nc = bacc.Bacc("TRN2", target_bir_lowering=False, debug=False,
                   num_devices=num_devices)

    dram = {}
    for nm, shape in INPUT_SPECS:
        dram[nm] = nc.declare_dram_parameter(nm, list(shape), f32, isOutput=False)
    y = nc.declare_dram_parameter("y", [1, 256], f32, isOutput=True)

    with tile.TileContext(nc) as tc:
        with contextlib.ExitStack() as ctx:
            const = ctx.enter_context(tc.tile_pool(name="const", bufs=1))
            work = ctx.enter_context(tc.tile_pool(name="work", bufs=2))
            upool = ctx.enter_context(tc.tile_pool(name="upool", bufs=2))
            gpool = ctx.enter_context(tc.tile_pool(name="gpool", bufs=2))
            pps = ctx.enter_context(tc.tile_pool(name="pps", bufs=2, space="PSUM"))
            spsum = ctx.enter_context(tc.tile_pool(name="spsum", bufs=3,
                                                   space="PSUM"))

            sb = {}
            for nm, shape in INPUT_SPECS:
                t = const.tile(list(shape), f32, name=f"sb_{nm}", tag=f"sb_{nm}")
                nc.gpsimd.dma_start(out=t[:], in_=dram[nm][:])
                sb[nm] = t

            ones128 = const.tile([1, 128], f32, name="ones128", tag="ones128")
            nc.vector.memset(ones128[:], 1.0)

            def emit_dbg(src_ap):
                dbg = work.tile([1, 256], f32, name="dbg", tag="dbg")
                nc.vector.tensor_copy(dbg[:], src_ap)
                nc.gpsimd.dma_start(out=y[:], in_=dbg[:])

            if stage < 1:
                emit_dbg(sb["cls_loc"][:])

            if stage >= 1:
                # ---------- i-side preprocessing (global sorted order) ------
                ps_f = pps.tile([64, N], f32, name="ps_f", tag="ps")
                nc.tensor.matmul(ps_f[:], sb["W_cls"][:], sb["bfT_i"][:],
                                 start=True, stop=True)
                featsT_i = const.tile([64, N], f32, name="featsT_i",
                                      tag="featsT_i")
                nc.vector.tensor_scalar(featsT_i[:], ps_f[:], sb["bcls"][:],
                                        0.0, OP.add, OP.max)

                ps_A = pps.tile([128, N], f32, name="ps_A", tag="ps")
                nc.tensor.matmul(ps_A[:], sb["W_in"][:], featsT_i[:],
                                 start=True, stop=False)
                nc.tensor.matmul(ps_A[:], sb["W_pos"][:], sb["posT_i"][:],
                                 start=False, stop=True)
                A_T = const.tile([128, N], f32, name="A_T", tag="A_T")
                nc.vector.tensor_scalar_add(A_T[:], ps_A[:], sb["bpos"][:])

                ps_p = pps.tile([128, N], f32, name="ps_p", tag="ps")
                nc.tensor.matmul(ps_p[:], sb["W_e1"][:], A_T[:],
                                 start=True, stop=True)
                p_T = const.tile([128, N], f32, name="p_T", tag="p_T")
                nc.vector.tensor_copy(p_T[:], ps_p[:])

                # ---------- j-side preprocessing (core-local j order) -------
                ps_fj = pps.tile([64, 256], f32, name="ps_fj", tag="ps")
                nc.tensor.matmul(ps_fj[:], sb["W_cls"][:], sb["bfT_j"][:],
                                 start=True, stop=True)
                featsT_j = const.tile([64, 256], f32, name="featsT_j",
                                      tag="featsT_j")
                nc.vector.tensor_scalar(featsT_j[:], ps_fj[:], sb["bcls"][:],
                                        0.0, OP.add, OP.max)

                ps_C = pps.tile([128, 256], f32, name="ps_C", tag="ps")
                nc.tensor.matmul(ps_C[:], sb["W_out"][:], featsT_j[:],
                                 start=True, stop=False)
                nc.tensor.matmul(ps_C[:], sb["W_pos"][:], sb["posT_j"][:],
                                 start=False, stop=True)
                C_T = const.tile([128, 256], f32, name="C_T", tag="C_T")
                nc.vector.tensor_copy(C_T[:], ps_C[:])

                ps_q = pps.tile([128, 256], f32, name="ps_q", tag="ps")
                nc.tensor.matmul(ps_q[:], sb["W_e1"][:], C_T[:],
                                 start=True, stop=True)
                qneg = const.tile([128, 256], f32, name="qneg", tag="qneg")
                nc.vector.tensor_scalar(qneg[:], ps_q[:], -1.0, sb["be1"][:],
                                        OP.mult, OP.add)

                if stage < 2:
                    emit_dbg(p_T[0:1, :256])

            if stage >= 2:
                # ---------- suppression masks -------------------------------
                ps_ab = pps.tile([128, N], f32, name="ps_ab", tag="ps")
                nc.tensor.matmul(ps_ab[:], ones128[:], sb["angrow"][:],
                                 start=True, stop=True)
                angb = const.tile([128, N], f32, name="angb", tag="angb")
                nc.vector.tensor_copy(angb[:], ps_ab[:])

                ps_io = pps.tile([128, N], f32, name="ps_io", tag="ps")
                nc.tensor.matmul(ps_io[:], ones128[:], sb["iota"][:],
                                 start=True, stop=True)
                iotab = const.tile([128, N], f32, name="iotab", tag="iotab")
                nc.vector.tensor_copy(iotab[:], ps_io[:])

                masks = []
                for g in range(2):
                    Lg = 256 if g == 0 else 512
                    acol = sb["angcol"][:, g:g + 1]
                    m1 = work.tile([128, Lg], f32, name=f"m1_{g}", tag="mtmp1")
                    nc.vector.tensor_scalar(m1[:], angb[:, :Lg], acol, 0.5,
                                            OP.subtract, OP.is_lt)
                    m2 = work.tile([128, Lg], f32, name=f"m2_{g}", tag="mtmp2")
                    nc.vector.tensor_scalar(m2[:], angb[:, :Lg], acol, -0.5,
                                            OP.subtract, OP.is_gt)
                    tri = work.tile([128, Lg], f32, name=f"tri_{g}", tag="mtmp3")
                    nc.vector.tensor_scalar(tri[:], iotab[:, :Lg],
                                            sb["rankcol"][:, g:g + 1], None,
                                            OP.is_lt)
                    t3 = work.tile([128, Lg], f32, name=f"t3_{g}", tag="mtmp1")
                    nc.vector.tensor_tensor(t3[:], m1[:], m2[:], OP.logical_and)
                    mg = const.tile([128, Lg], f32, name=f"mask{g}",
                                    tag=f"mask{g}")
                    nc.vector.tensor_tensor(mg[:], t3[:], tri[:], OP.logical_and)
                    masks.append(mg)

                if stage < 3:
                    emit_dbg(masks[1][0:1, :256])

            if stage >= 3:
                # ---------- main loop ---------------------------------------
                nmall = const.tile([TJ, N_TILES], f32, name="nmall", tag="nmall")
                n_tiles_run = 1 if stage == 3 else N_TILES
                if stage == 3:
                    nc.vector.memset(nmall[:], 0.0)
                for t in range(n_tiles_run):
                    L = ILEN[t]
                    g, prow = t // 4, TJ * (t % 4)
                    S = spsum.tile([TJ, L], f32, name=f"S_{t}", tag="sbank")
                    for c in range(2):
                        U = upool.tile([128, 16 * L], f32, name=f"U_{t}_{c}",
                                       tag="u")
                        for jj in range(16):
                            l = TJ * t + 16 * c + jj
                            nc.vector.tensor_scalar_add(
                                U[:, jj * L:(jj + 1) * L], p_T[:, :L],
                                qneg[:, l:l + 1])
                        G = gpool.tile([128, 16 * L], f32, name=f"G_{t}_{c}",
                                       tag="g")
                        nc.scalar.activation(G[:], U[:], AF.Gelu)
                        for jj in range(16):
                            r = 16 * c + jj
                            nc.tensor.matmul(S[:, :],
                                             sb["we2d"][:, TJ * r:TJ * (r + 1)],
                                             G[:, jj * L:(jj + 1) * L],
                                             start=(r == 0), stop=(r == TJ - 1))
                    # masked = (S + b_e2) * mask ; node_max = rowmax(masked)
                    msk = work.tile([TJ, L], f32, name=f"msk_{t}", tag="msk")
                    nc.vector.scalar_tensor_tensor(
                        msk[:], S[:], sb["be2c"][prow:prow + TJ],
                        masks[g][prow:prow + TJ, :L], OP.add, OP.mult)
                    nc.vector.reduce_max(nmall[:, t:t + 1], msk[:], axis=AX.X)

                if stage < 5:
                    fl = work.tile([1, 256], f32, name="fl", tag="dbg")
                    nc.gpsimd.dma_start(out=fl[:], in_=nmall[:])
                    nc.gpsimd.dma_start(out=y[:], in_=fl[:])

            if stage >= 5:
                # ---------- final MLP over node_max -------------------------
                # flatten [32, 8] -> [1, 256]; f = 8*pp + q (host unscrambles)
                nm_flat = work.tile([1, 256], f32, name="nm_flat", tag="nm_flat")
                nc.gpsimd.dma_start(out=nm_flat[:], in_=nmall[:])

                ps_h1 = pps.tile([64, 256], f32, name="ps_h1", tag="ps")
                nc.tensor.matmul(ps_h1[:], sb["W_n1"][:], nm_flat[:],
                                 start=True, stop=True)
                s1 = work.tile([64, 256], f32, name="s1", tag="s1")
                nc.vector.tensor_scalar(s1[:], ps_h1[:], sb["bn1"][:], 0.0,
                                        OP.add, OP.max)

                ps_h2 = pps.tile([64, 256], f32, name="ps_h2", tag="ps")
                nc.tensor.matmul(ps_h2[:], sb["W_n2"][:], s1[:],
                                 start=True, stop=True)
                s2 = work.tile([64, 256], f32, name="s2", tag="s2")
                nc.vector.tensor_scalar(s2[:], ps_h2[:], sb["bn2"][:], 0.0,
                                        OP.add, OP.max)

                ps_L0 = pps.tile([1, 256], f32, name="ps_L0", tag="ps")
                nc.tensor.matmul(ps_L0[:], sb["W_head"][:], s2[:],
                                 start=True, stop=True)
                t1 = work.tile([1, 256], f32, name="t1f", tag="t1f")
                nc.vector.tensor_scalar(t1[:], ps_L0[:], sb["bh"][:], 1.0e6,
                                        OP.add, OP.add)
                mker = work.tile([1, 256], f32, name="mker", tag="mker")
                nc.vector.tensor_scalar(mker[:], sb["cls_loc"][:],
                                        float(F32(CONF_THRES)), None, OP.is_ge)
                t2 = work.tile([1, 256], f32, name="t2f", tag="t2f")
                nc.vector.tensor_tensor(t2[:], t1[:], mker[:], OP.mult)
                t3f = work.tile([1, 256], f32, name="t3f", tag="t3f")
                nc.vector.tensor_scalar_add(t3f[:], t2[:], -1.0e6)
                out_t = work.tile([1, 256], f32, name="out_t", tag="out_t")
                nc.scalar.activation(out_t[:], t3f[:], AF.Sigmoid)
                nc.gpsimd.dma_start(out=y[:], in_=out_t[:])

    nc.compile()
    return nc


def _get_program():
    global _PROGRAM
    if _PROGRAM is None:
        _PROGRAM = _build_program()
    return _PROGRAM


def _pos_emb(e0, e1):
    """float32 mirror of the reference _get_sample_point (one batch, sorted)."""
    angle = (e0 * F32(np.pi)).astype(F32)
    rho = (e1 * F32(IMG_W)).astype(F32)
    lin = np.linspace(0.0, 1.0 - 1e-5, NUM_OFFSETS, dtype=F32)
    yk = (F32(CENTER_H) - lin * F32(IMG_H)).astype(F32)[:2]
    tan = np.tan(angle, dtype=F32)
    roc = (rho / np.cos(angle, dtype=F32)).astype(F32)
    x = (-tan[:, None] * yk[None, :] + roc[:, None]).astype(F32)
    return (x / F32(IMG_W)).astype(F32)          # [n, 2]


def kernel(**inputs):
    bf = np.asarray(inputs["batch_features"], dtype=F32)      # [B,N,64]
    cls = np.asarray(inputs["cls_pred"], dtype=F32)           # [B,N]
    aid = np.asarray(inputs["anchor_id"])                     # [B,N] int32
    emb = np.asarray(inputs["anchor_embeddings"], dtype=F32)  # [B,N,2]

    w = {k: np.asarray(inputs[k], dtype=F32) for k in
         ("W_cls", "b_cls", "W_pos", "b_pos", "W_in", "b_in", "W_out", "b_out",
          "W_e1", "b_e1", "W_e2", "b_e2", "W_n1", "b_n1", "W_n2", "b_n2",
          "W_head", "b_head")}
    # A = feats@W_in + pos@W_pos + (b_in + b_pos); C = feats@W_out + b_out
    # + pos@W_pos.  Device omits b_out in C; fold it into be1:
    # qneg = b_e1 - q = (b_e1 - b_out@W_e1) - (C - b_out)@W_e1.
    bpos_eff = (w["b_in"] + w["b_pos"]).astype(F32)
    be1_eff = (w["b_e1"] - w["b_out"] @ w["W_e1"]).astype(F32)

    nc = _get_program()
    from concourse.bass_utils import run_bass_kernel_spmd

    iota = np.arange(N, dtype=F32)[None, :]
    we2d = np.zeros((I_DIM, TJ * TJ), dtype=F32)
    for j in range(TJ):
        we2d[:, TJ * j + j] = w["W_e2"][:, 0]
    # device nm_flat order: f = 8*pp + q  <->  local j index l = 32*q + pp
    l_of_f = np.array([TJ * q + pp for pp in range(TJ) for q in range(N_TILES)])

    shared = {
        "iota": iota, "we2d": we2d,
        "W_cls": w["W_cls"], "bcls": w["b_cls"][:, None],
        "W_in": w["W_in"], "W_out": w["W_out"], "W_pos": w["W_pos"],
        "bpos": bpos_eff[:, None], "W_e1": w["W_e1"],
        "be1": be1_eff[:, None], "we2": w["W_e2"],
        "be2c": np.full((128, 1), w["b_e2"][0], dtype=F32),
        "W_n1": w["W_n1"], "bn1": w["b_n1"][:, None],
        "W_n2": w["W_n2"], "bn2": w["b_n2"][:, None],
        "W_head": w["W_head"], "bh": w["b_head"][:, None],
    }

    in_maps = []
    perms = []
    rank_lists = []
    for b in range(B):
        perm = np.lexsort((-aid[b].astype(np.int64), -cls[b]))
        perms.append(perm)
        bf_s = bf[b][perm]                    # [N, 64]
        cls_s = cls[b][perm]
        e0_s = emb[b][perm, 0]
        e1_s = emb[b][perm, 1]
        ang_s = (e0_s * F32(np.pi)).astype(F32)
        pos_s = _pos_emb(e0_s, e1_s)          # [N, 2]

        bfT_i = np.ascontiguousarray(bf_s.T)
        posT_i = np.ascontiguousarray(pos_s.T)

        for P in range(2):
            ranks = np.concatenate(
                [np.arange(TJ * (2 * t + P), TJ * (2 * t + P) + TJ)
                 for t in range(N_TILES)])
            rank_lists.append(ranks[l_of_f])
            ang_loc = ang_s[ranks]
            m = dict(shared)
            m.update({
                "bfT_i": bfT_i,
                "posT_i": posT_i,
                "bfT_j": np.ascontiguousarray(bf_s[ranks].T),
                "posT_j": np.ascontiguousarray(pos_s[ranks].T),
                "angrow": ang_s[None, :],
                "angcol": np.ascontiguousarray(
                    np.stack([ang_loc[:128], ang_loc[128:]], axis=1)),
                "rankcol": np.ascontiguousarray(
                    np.stack([ranks[:128].astype(F32),
                              ranks[128:].astype(F32)], axis=1)),
                "cls_loc": cls_s[ranks[l_of_f]][None, :],
            })
            in_maps.append(m)

    res = run_bass_kernel_spmd(nc, in_maps, list(range(N_CORES)))

    out = np.zeros((B, N), dtype=F32)
    for ci in range(N_CORES):
        b = ci // 2
        probs = res.results[ci]["y"][0]       # [256] in core-local j order
        out[b, perms[b][rank_lists[ci]]] = probs
    return out



# revision 6
# speedup vs baseline: 3.9949x; 3.9949x over previous
"""Trainium2 Bass kernel for the O2O classification head (GNN message passing).

Strategy (v2)
-------------
The edge tensor is rank-structured: after the first edge-MLP layer the
pre-gelu value for pair (i, j) is u = p_i + qneg_j with per-node vectors
    p    = (feats@W_in + pos@W_pos + b_in + b_pos) @ W_e1
    qneg = b_e1 - (feats@W_out + pos@W_pos + b_out) @ W_e1
so the device only computes gelu(p_i + qneg_j) . W_e2 per pair.

Host-side, nodes are sorted by (cls desc, id desc); suppress[i,j] != 0
requires rank_i < rank_j.  Ranks with cls < 0.4 have output exactly
sigmoid(-1e6) = 0, and sorting puts them at ranks >= K, so only the
top-K block is computed at all (K = #{cls >= 0.4} rounded up to 16).

Sharding: 2 cores per batch; core parity P takes ranks == P (mod 2).
Per core, j's are processed in t-blocks of 8 with i-prefix 16(t+1).
All pair work is bf16: DVE broadcasts u = p + qneg (4x mode), Act does
gelu, PE dot-products with W_e2 accumulate into PSUM on top of a
host-precomputed additive mask (0 where allowed, -30000 elsewhere,
b_e2 folded in) injected via an identity-stationary matmul.  A row-max
per t-block then gives node_max; the tiny 64-wide output MLP and the
sigmoid run on host.
"""

import sys
import numpy as np
import ml_dtypes

if "/opt/trn_rl_repo" not in sys.path:
    sys.path.insert(0, "/opt/trn_rl_repo")

B, N = 4, 512
H_DIM, I_DIM = 64, 128
N_CORES = 8
TJ = 8               # j's per t-block
GROUP_W = 512        # max PSUM bank width (fp32 cols)
F32 = np.float32
BF16 = ml_dtypes.bfloat16

IMG_W, IMG_H, CENTER_H = 800.0, 320.0, 160.0
NUM_OFFSETS = 72
CONF_THRES = 0.4
MASK_NEG = -30000.0

_PROGRAMS = {}       # n_t -> compiled program
_LAST_NT = None


def _lens(n_t):
    return [16 * (t + 1) for t in range(n_t)]


def _groups(n_t):
    """Greedy grouping of consecutive t-blocks with sum(L) <= GROUP_W."""
    Ls = _lens(n_t)
    groups, cur, cur_w = [], [], 0
    for t in range(n_t):
        if cur and cur_w + Ls[t] > GROUP_W:
            groups.append((cur, cur_w))
            cur, cur_w = [], 0
        cur.append(t)
        cur_w += Ls[t]
    groups.append((cur, cur_w))
    return groups


def _build_program(n_t, num_devices=N_CORES):
    import contextlib
    import concourse.bass as bass  # noqa: F401
    import concourse.tile as tile
    from concourse import bacc, mybir

    f32 = mybir.dt.float32
    bf16 = mybir.dt.bfloat16
    AF = mybir.ActivationFunctionType
    AX = mybir.AxisListType

    Ls = _lens(n_t)
    groups = _groups(n_t)
    Lsum = sum(Ls)
    J = TJ * n_t
    K = 2 * J

    nc = bacc.Bacc("TRN2", target_bir_lowering=False, debug=False,
                   num_devices=num_devices)

    d_p16 = nc.declare_dram_parameter("p16", [I_DIM, K], bf16, isOutput=False)
    d_qneg = nc.declare_dram_parameter("qneg", [I_DIM, J], f32, isOutput=False)
    d_madd = nc.declare_dram_parameter("madd", [TJ, Lsum], bf16,
                                       isOutput=False)
    d_we2 = nc.declare_dram_parameter("we2", [I_DIM, TJ * TJ], bf16,
                                      isOutput=False)
    d_i8 = nc.declare_dram_parameter("i8", [TJ, TJ], bf16, isOutput=False)
    y = nc.declare_dram_parameter("y", [TJ, n_t], f32, isOutput=True)

    with tile.TileContext(nc) as tc:
        with contextlib.ExitStack() as ctx:
            const = ctx.enter_context(tc.tile_pool(name="const", bufs=1))
            ub = ctx.enter_context(tc.tile_pool(name="ub", bufs=2))
            gb = ctx.enter_context(tc.tile_pool(name="gb", bufs=2))
            sp = ctx.enter_context(tc.tile_pool(name="sp", bufs=3,
                                                space="PSUM"))

            p16 = const.tile([I_DIM, K], bf16, name="p16", tag="p16")
            qneg = const.tile([I_DIM, J], f32, name="qneg", tag="qneg")
            madd = const.tile([TJ, Lsum], bf16, name="madd", tag="madd")
            we2 = const.tile([I_DIM, TJ * TJ], bf16, name="we2", tag="we2")
            i8 = const.tile([TJ, TJ], bf16, name="i8", tag="i8")
            nc.gpsimd.dma_start(out=p16[:], in_=d_p16[:])
            nc.gpsimd.dma_start(out=qneg[:], in_=d_qneg[:])
            nc.gpsimd.dma_start(out=madd[:], in_=d_madd[:])
            nc.gpsimd.dma_start(out=we2[:], in_=d_we2[:])
            nc.gpsimd.dma_start(out=i8[:], in_=d_i8[:])

            nmall = const.tile([TJ, n_t], f32, name="nmall", tag="nmall")

            UW = TJ * GROUP_W
            pending = []

            def flush_one():
                S, ts, offs, w = pending.pop(0)
                for t, off in zip(ts, offs):
                    nc.vector.reduce_max(nmall[:, t:t + 1],
                                         S[:, off:off + Ls[t]], axis=AX.X)

            goff = 0
            for gi, (ts, w) in enumerate(groups):
                U = ub.tile([I_DIM, UW], bf16, name=f"U{gi}", tag="U")
                G = gb.tile([I_DIM, UW], bf16, name=f"G{gi}", tag="G")
                off = 0
                offs = []
                for t in ts:
                    L = Ls[t]
                    for jj in range(TJ):
                        nc.vector.tensor_scalar_add(
                            U[:, jj * w + off: jj * w + off + L],
                            p16[:, :L],
                            qneg[:, TJ * t + jj: TJ * t + jj + 1])
                    offs.append(off)
                    off += L
                # gelu in two chunks (jj 0-3, 4-7) for pipeline granularity
                half = (TJ // 2) * w
                nc.scalar.activation(G[:, :half], U[:, :half], AF.Gelu)
                nc.scalar.activation(G[:, half:TJ * w], U[:, half:TJ * w],
                                     AF.Gelu)

                S = sp.tile([TJ, GROUP_W], f32, name=f"S{gi}", tag="S")
                nc.tensor.matmul(S[:, :w], i8[:], madd[:, goff:goff + w],
                                 start=True, stop=False)
                for jj in range(TJ):
                    nc.tensor.matmul(S[:, :w], we2[:, TJ * jj: TJ * (jj + 1)],
                                     G[:, jj * w: (jj + 1) * w],
                                     start=False, stop=(jj == TJ - 1))
                pending.append((S, ts, offs, w))
                goff += w
                if len(pending) > 2:
                    flush_one()

            while pending:
                flush_one()

            nc.gpsimd.dma_start(out=y[:], in_=nmall[:])

    nc.compile()
    return nc


def _get_program(n_t=None):
    global _LAST_NT
    if n_t is None:
        n_t = _LAST_NT
    if n_t not in _PROGRAMS:
        _PROGRAMS[n_t] = _build_program(n_t)
    _LAST_NT = n_t
    return _PROGRAMS[n_t]


def _pos_emb(e0, e1):
    """float32 mirror of the reference _get_sample_point (one batch, sorted)."""
    angle = (e0 * F32(np.pi)).astype(F32)
    rho = (e1 * F32(IMG_W)).astype(F32)
    lin = np.linspace(0.0, 1.0 - 1e-5, NUM_OFFSETS, dtype=F32)
    yk = (F32(CENTER_H) - lin * F32(IMG_H)).astype(F32)[:2]
    tan = np.tan(angle, dtype=F32)
    roc = (rho / np.cos(angle, dtype=F32)).astype(F32)
    x = (-tan[:, None] * yk[None, :] + roc[:, None]).astype(F32)
    return (x / F32(IMG_W)).astype(F32)          # [n, 2]


def kernel(**inputs):
    global _LAST_NT
    bf = np.asarray(inputs["batch_features"], dtype=F32)      # [B,N,64]
    cls = np.asarray(inputs["cls_pred"], dtype=F32)           # [B,N]
    aid = np.asarray(inputs["anchor_id"])                     # [B,N] int32
    emb = np.asarray(inputs["anchor_embeddings"], dtype=F32)  # [B,N,2]

    w = {k: np.asarray(inputs[k], dtype=F32) for k in
         ("W_cls", "b_cls", "W_pos", "b_pos", "W_in", "b_in", "W_out", "b_out",
          "W_e1", "b_e1", "W_e2", "b_e2", "W_n1", "b_n1", "W_n2", "b_n2",
          "W_head", "b_head")}

    out = np.zeros((B, N), dtype=F32)

    perms, Kbs = [], []
    for b in range(B):
        perm = np.lexsort((-aid[b].astype(np.int64), -cls[b]))
        perms.append(perm)
        Kbs.append(int((cls[b] >= F32(CONF_THRES)).sum()))
    Kmax = max(Kbs)
    if Kmax == 0:
        return out

    K = min(N, 16 * ((Kmax + 15) // 16))
    n_t = K // 16
    J = K // 2
    Ls = _lens(n_t)
    groups = _groups(n_t)
    Lsum = sum(Ls)

    nc = _get_program(n_t)
    from concourse.bass_utils import run_bass_kernel_spmd

    be2 = float(w["b_e2"][0])
    we2d = np.zeros((I_DIM, TJ * TJ), dtype=F32)              # one-hot blocks
    for jj in range(TJ):
        we2d[:, TJ * jj + jj] = w["W_e2"][:, 0]
    we2_16 = we2d.astype(BF16)
    i8 = np.eye(TJ, dtype=BF16)

    in_maps = []
    for b in range(B):
        perm = perms[b]
        bf_s = bf[b][perm][:K]                 # [K, 64]
        e0_s = emb[b][perm, 0][:K]
        e1_s = emb[b][perm, 1][:K]
        ang_s = (e0_s * F32(np.pi)).astype(F32)
        pos_s = _pos_emb(e0_s, e1_s)           # [K, 2]

        feats = np.maximum(bf_s @ w["W_cls"] + w["b_cls"], 0.0).astype(F32)
        base = (pos_s @ w["W_pos"]).astype(F32)
        A = (feats @ w["W_in"] + base + (w["b_in"] + w["b_pos"])).astype(F32)
        C = (feats @ w["W_out"] + base + w["b_out"]).astype(F32)
        p = (A @ w["W_e1"]).astype(F32)                        # [K, 128]
        qneg_full = (w["b_e1"] - C @ w["W_e1"]).astype(F32)    # [K, 128]

        p16 = np.ascontiguousarray(p.T).astype(BF16)           # [128, K]

        iota = np.arange(K)
        for P in range(2):
            ranks = 2 * np.arange(J) + P                       # [J]
            qneg = np.ascontiguousarray(qneg_full[ranks].T)    # [128, J] f32
            # allowed[c, i] = |ang_i - ang_rank_c| < 0.5  and  i < rank_c
            dif = np.abs(ang_s[None, :] - ang_s[ranks][:, None]).astype(F32)
            allowed = (dif < F32(0.5)) & (iota[None, :] < ranks[:, None])
            madd = np.full((TJ, Lsum), MASK_NEG, dtype=F32)
            goff = 0
            for ts, wg in groups:
                off = 0
                for t in ts:
                    L = Ls[t]
                    rows = allowed[TJ * t: TJ * t + TJ, :L]
                    blk = np.where(rows, be2, MASK_NEG)
                    madd[:, goff + off: goff + off + L] = blk
                    off += L
                goff += wg
            in_maps.append({
                "p16": p16,
                "qneg": qneg,
                "madd": madd.astype(BF16),
                "we2": we2_16,
                "i8": i8,
            })

    res = run_bass_kernel_spmd(nc, in_maps, list(range(N_CORES)))

    for ci in range(N_CORES):
        b, P = ci // 2, ci % 2
        ymat = np.asarray(res.results[ci]["y"], dtype=F32)     # [TJ, n_t]
        nm = np.maximum(ymat.T.reshape(-1), 0.0).astype(F32)   # [J] c=8t+jj
        s1 = np.maximum(nm[:, None] * w["W_n1"][0][None, :] + w["b_n1"],
                        0.0).astype(F32)
        s2 = np.maximum(s1 @ w["W_n2"] + w["b_n2"], 0.0).astype(F32)
        logit = (s2 @ w["W_head"][:, 0] + w["b_head"][0]).astype(F32)
        prob = (1.0 / (1.0 + np.exp(-logit.astype(np.float64)))).astype(F32)
        ranks = 2 * np.arange(J) + P
        valid = ranks < Kbs[b]
        out[b, perms[b][ranks[valid]]] = prob[valid]
    return out


# revision 15
# speedup vs baseline: 4.0371x; 1.0106x over previous
"""Trainium2 Bass kernel for the O2O classification head (GNN message passing).

Strategy (v2)
-------------
The edge tensor is rank-structured: after the first edge-MLP layer the
pre-gelu value for pair (i, j) is u = p_i + qneg_j with per-node vectors
    p    = (feats@W_in + pos@W_pos + b_in + b_pos) @ W_e1
    qneg = b_e1 - (feats@W_out + pos@W_pos + b_out) @ W_e1
so the device only computes gelu(p_i + qneg_j) . W_e2 per pair.

Host-side, nodes are sorted by (cls desc, id desc); suppress[i,j] != 0
requires rank_i < rank_j.  Ranks with cls < 0.4 have output exactly
sigmoid(-1e6) = 0, and sorting puts them at ranks >= K, so only the
top-K block is computed at all (K = #{cls >= 0.4} rounded up to 16).

Sharding: 2 cores per batch; core parity P takes ranks == P (mod 2).
Per core, j's are processed in t-blocks of 8 with i-prefix 16(t+1).
All pair work is bf16: DVE broadcasts u = p + qneg (4x mode), Act does
gelu, PE dot-products with W_e2 accumulate into PSUM on top of a
host-precomputed additive mask (0 where allowed, -30000 elsewhere,
b_e2 folded in) injected via an identity-stationary matmul.  A row-max
per t-block then gives node_max; the tiny 64-wide output MLP and the
sigmoid run on host.
"""

import sys
import numpy as np
import ml_dtypes

if "/opt/trn_rl_repo" not in sys.path:
    sys.path.insert(0, "/opt/trn_rl_repo")

B, N = 4, 512
H_DIM, I_DIM = 64, 128
N_CORES = 8
TJ = 8               # j's per t-block
GROUP_W = 512        # max PSUM bank width (fp32 cols)
F32 = np.float32
BF16 = ml_dtypes.bfloat16

IMG_W, IMG_H, CENTER_H = 800.0, 320.0, 160.0
NUM_OFFSETS = 72
CONF_THRES = 0.4
MASK_NEG = -30000.0

_PROGRAMS = {}       # n_t -> compiled program
_LAST_NT = None


def _lens(n_t):
    return [16 * (t + 1) for t in range(n_t)]


def _groups(n_t):
    """Greedy grouping of consecutive t-blocks with sum(L) <= GROUP_W."""
    Ls = _lens(n_t)
    groups, cur, cur_w = [], [], 0
    for t in range(n_t):
        if cur and cur_w + Ls[t] > GROUP_W:
            groups.append((cur, cur_w))
            cur, cur_w = [], 0
        cur.append(t)
        cur_w += Ls[t]
    groups.append((cur, cur_w))
    return groups


def _build_program(n_t, num_devices=N_CORES):
    import contextlib
    import concourse.bass as bass  # noqa: F401
    import concourse.tile as tile
    from concourse import bacc, mybir

    f32 = mybir.dt.float32
    bf16 = mybir.dt.bfloat16
    AF = mybir.ActivationFunctionType
    AX = mybir.AxisListType

    Ls = _lens(n_t)
    groups = _groups(n_t)
    Lsum = sum(Ls)
    J = TJ * n_t
    K = 2 * J

    nc = bacc.Bacc("TRN2", target_bir_lowering=False, debug=False,
                   num_devices=num_devices)

    d_p16 = nc.declare_dram_parameter("p16", [I_DIM, K], bf16, isOutput=False)
    d_qneg = nc.declare_dram_parameter("qneg", [I_DIM, J], f32, isOutput=False)
    d_madd = nc.declare_dram_parameter("madd", [TJ, Lsum], bf16,
                                       isOutput=False)
    d_we2 = nc.declare_dram_parameter("we2", [I_DIM, TJ * TJ], bf16,
                                      isOutput=False)
    d_i8 = nc.declare_dram_parameter("i8", [TJ, TJ], bf16, isOutput=False)
    y = nc.declare_dram_parameter("y", [TJ, n_t], f32, isOutput=True)

    with tile.TileContext(nc) as tc:
        with contextlib.ExitStack() as ctx:
            const = ctx.enter_context(tc.tile_pool(name="const", bufs=1))
            ub = ctx.enter_context(tc.tile_pool(name="ub", bufs=2))
            gb = ctx.enter_context(tc.tile_pool(name="gb", bufs=2))
            sp = ctx.enter_context(tc.tile_pool(name="sp", bufs=3,
                                                space="PSUM"))

            p16 = const.tile([I_DIM, K], bf16, name="p16", tag="p16")
            qneg = const.tile([I_DIM, J], f32, name="qneg", tag="qneg")
            madd = const.tile([TJ, Lsum], bf16, name="madd", tag="madd")
            we2 = const.tile([I_DIM, TJ * TJ], bf16, name="we2", tag="we2")
            i8 = const.tile([TJ, TJ], bf16, name="i8", tag="i8")
            nc.gpsimd.dma_start(out=p16[:], in_=d_p16[:])
            nc.gpsimd.dma_start(out=qneg[:], in_=d_qneg[:])
            nc.gpsimd.dma_start(out=madd[:], in_=d_madd[:])
            nc.gpsimd.dma_start(out=we2[:], in_=d_we2[:])
            nc.gpsimd.dma_start(out=i8[:], in_=d_i8[:])

            nmall = const.tile([TJ, n_t], f32, name="nmall", tag="nmall")

            UW = TJ * GROUP_W
            pending = []

            def flush_one():
                S, ts, offs, w = pending.pop(0)
                for t, off in zip(ts, offs):
                    nc.vector.reduce_max(nmall[:, t:t + 1],
                                         S[:, off:off + Ls[t]], axis=AX.X)

            goff = 0
            for gi, (ts, w) in enumerate(groups):
                U = ub.tile([I_DIM, UW], bf16, name=f"U{gi}", tag="U")
                G = gb.tile([I_DIM, UW], bf16, name=f"G{gi}", tag="G")
                off = 0
                offs = []
                for t in ts:
                    L = Ls[t]
                    for jj in range(TJ):
                        nc.vector.tensor_scalar_add(
                            U[:, jj * w + off: jj * w + off + L],
                            p16[:, :L],
                            qneg[:, TJ * t + jj: TJ * t + jj + 1])
                    offs.append(off)
                    off += L
                # gelu in two chunks (jj 0-3, 4-7) for pipeline granularity
                half = (TJ // 2) * w
                nc.scalar.activation(G[:, :half], U[:, :half], AF.Gelu)
                nc.scalar.activation(G[:, half:TJ * w], U[:, half:TJ * w],
                                     AF.Gelu)

                S = sp.tile([TJ, GROUP_W], f32, name=f"S{gi}", tag="S")
                nc.tensor.matmul(S[:, :w], i8[:], madd[:, goff:goff + w],
                                 start=True, stop=False)
                for jj in range(TJ):
                    nc.tensor.matmul(S[:, :w], we2[:, TJ * jj: TJ * (jj + 1)],
                                     G[:, jj * w: (jj + 1) * w],
                                     start=False, stop=(jj == TJ - 1))
                pending.append((S, ts, offs, w))
                goff += w
                if len(pending) > 1:
                    flush_one()

            while pending:
                flush_one()

            nc.gpsimd.dma_start(out=y[:], in_=nmall[:])

    nc.compile()
    return nc


def _get_program(n_t=None):
    global _LAST_NT
    if n_t is None:
        n_t = _LAST_NT
    if n_t not in _PROGRAMS:
        _PROGRAMS[n_t] = _build_program(n_t)
    _LAST_NT = n_t
    return _PROGRAMS[n_t]


def _pos_emb(e0, e1):
    """float32 mirror of the reference _get_sample_point (one batch, sorted)."""
    angle = (e0 * F32(np.pi)).astype(F32)
    rho = (e1 * F32(IMG_W)).astype(F32)
    lin = np.linspace(0.0, 1.0 - 1e-5, NUM_OFFSETS, dtype=F32)
    yk = (F32(CENTER_H) - lin * F32(IMG_H)).astype(F32)[:2]
    tan = np.tan(angle, dtype=F32)
    roc = (rho / np.cos(angle, dtype=F32)).astype(F32)
    x = (-tan[:, None] * yk[None, :] + roc[:, None]).astype(F32)
    return (x / F32(IMG_W)).astype(F32)          # [n, 2]


def kernel(**inputs):
    global _LAST_NT
    bf = np.asarray(inputs["batch_features"], dtype=F32)      # [B,N,64]
    cls = np.asarray(inputs["cls_pred"], dtype=F32)           # [B,N]
    aid = np.asarray(inputs["anchor_id"])                     # [B,N] int32
    emb = np.asarray(inputs["anchor_embeddings"], dtype=F32)  # [B,N,2]

    w = {k: np.asarray(inputs[k], dtype=F32) for k in
         ("W_cls", "b_cls", "W_pos", "b_pos", "W_in", "b_in", "W_out", "b_out",
          "W_e1", "b_e1", "W_e2", "b_e2", "W_n1", "b_n1", "W_n2", "b_n2",
          "W_head", "b_head")}

    out = np.zeros((B, N), dtype=F32)

    perms, Kbs = [], []
    for b in range(B):
        perm = np.lexsort((-aid[b].astype(np.int64), -cls[b]))
        perms.append(perm)
        Kbs.append(int((cls[b] >= F32(CONF_THRES)).sum()))
    Kmax = max(Kbs)
    if Kmax == 0:
        return out

    K = min(N, 16 * ((Kmax + 15) // 16))
    n_t = K // 16
    J = K // 2
    Ls = _lens(n_t)
    groups = _groups(n_t)
    Lsum = sum(Ls)

    nc = _get_program(n_t)
    from concourse.bass_utils import run_bass_kernel_spmd

    be2 = float(w["b_e2"][0])
    we2d = np.zeros((I_DIM, TJ * TJ), dtype=F32)              # one-hot blocks
    for jj in range(TJ):
        we2d[:, TJ * jj + jj] = w["W_e2"][:, 0]
    we2_16 = we2d.astype(BF16)
    i8 = np.eye(TJ, dtype=BF16)

    in_maps = []
    for b in range(B):
        perm = perms[b]
        bf_s = bf[b][perm][:K]                 # [K, 64]
        e0_s = emb[b][perm, 0][:K]
        e1_s = emb[b][perm, 1][:K]
        ang_s = (e0_s * F32(np.pi)).astype(F32)
        pos_s = _pos_emb(e0_s, e1_s)           # [K, 2]

        feats = np.maximum(bf_s @ w["W_cls"] + w["b_cls"], 0.0).astype(F32)
        base = (pos_s @ w["W_pos"]).astype(F32)
        A = (feats @ w["W_in"] + base + (w["b_in"] + w["b_pos"])).astype(F32)
        C = (feats @ w["W_out"] + base + w["b_out"]).astype(F32)
        p = (A @ w["W_e1"]).astype(F32)                        # [K, 128]
        qneg_full = (w["b_e1"] - C @ w["W_e1"]).astype(F32)    # [K, 128]

        p16 = np.ascontiguousarray(p.T).astype(BF16)           # [128, K]

        iota = np.arange(K)
        for P in range(2):
            ranks = 2 * np.arange(J) + P                       # [J]
            qneg = np.ascontiguousarray(qneg_full[ranks].T)    # [128, J] f32
            # allowed[c, i] = |ang_i - ang_rank_c| < 0.5  and  i < rank_c
            dif = np.abs(ang_s[None, :] - ang_s[ranks][:, None]).astype(F32)
            allowed = (dif < F32(0.5)) & (iota[None, :] < ranks[:, None])
            madd = np.full((TJ, Lsum), MASK_NEG, dtype=F32)
            goff = 0
            for ts, wg in groups:
                off = 0
                for t in ts:
                    L = Ls[t]
                    rows = allowed[TJ * t: TJ * t + TJ, :L]
                    blk = np.where(rows, be2, MASK_NEG)
                    madd[:, goff + off: goff + off + L] = blk
                    off += L
                goff += wg
            in_maps.append({
                "p16": p16,
                "qneg": qneg,
                "madd": madd.astype(BF16),
                "we2": we2_16,
                "i8": i8,
            })

    res = run_bass_kernel_spmd(nc, in_maps, list(range(N_CORES)))

    for ci in range(N_CORES):
        b, P = ci // 2, ci % 2
        ymat = np.asarray(res.results[ci]["y"], dtype=F32)     # [TJ, n_t]
        nm = np.maximum(ymat.T.reshape(-1), 0.0).astype(F32)   # [J] c=8t+jj
        s1 = np.maximum(nm[:, None] * w["W_n1"][0][None, :] + w["b_n1"],
                        0.0).astype(F32)
        s2 = np.maximum(s1 @ w["W_n2"] + w["b_n2"], 0.0).astype(F32)
        logit = (s2 @ w["W_head"][:, 0] + w["b_head"][0]).astype(F32)
        prob = (1.0 / (1.0 + np.exp(-logit.astype(np.float64)))).astype(F32)
        ranks = 2 * np.arange(J) + P
        valid = ranks < Kbs[b]
        out[b, perms[b][ranks[valid]]] = prob[valid]
    return out
